# revision 41
# baseline (speedup 1.0000x reference)
"""Trainium2 Bass kernel: topo-batched masked-norm NN forward (gnn_message_passing).

Math per topo batch i (reference.py):
    vals = previous layer activations [W]
    n_in[r]  = sum_c M[r,c]
    mean[r]  = (M @ vals)[r] / n_in[r]
    var[r]   = (M @ vals^2)[r] / n_in[r] - mean[r]^2
    rs[r]    = 1/sqrt(var[r] + EPS)
    affine[r]= gamma*rs*( (WM @ vals)[r] - mean[r]*rowWM[r] ) + beta[r]*rowWM[r]
               + bias[r],   WM = W ⊙ M,  rowWM = WM @ 1
    out = silu(affine*gain)*amp   (last batch: identity instead of silu)

Distribution: rows (output neurons) sharded across 8 cores (512 rows/core);
the 4096-vector of activations is all-gathered between batches.

Key layout/precision choices (vs the f32-accurate hi/lo variant):
  * ONE bf16 matrix per batch: WM = (W ⊙ M) premasked on host and shipped
    bf16 ([NB,128,CB,RPC], transposed: contraction dim on partitions).
    Halves HBM traffic vs hi/lo pairs + u8 masks; rel-err ~5.5e-3 vs the
    2e-2 gate (validated in numpy against the jax reference).
  * The 0/1 mask M is re-derived ON DEVICE as (WM != 0) on the DVE — exact,
    because no masked Gaussian weight rounds to bf16 zero, and the host
    writes +0.0 at masked-out entries. This keeps the gpsimd queue free
    (u8->bf16 converting DMA would require the slow software DGE, which
    delayed the collective trigger in the previous version).
  * Activations keep a bf16 hi/lo split (stationary side of the matmul is
    free: per-c-block cost is the 512 moving cols, not stationary rows).
  * n_in, 1/n_in, rowWM and the affine constants are input-independent =>
    precomputed on host and shipped as per-row params:
      P0 = gamma*gain,  P1 = (bias + beta*rowWM)*gain,  amp, rn=1/n_in, rowWM
    so the epilogue is ~15 DVE ops + one Silu ACT op per batch.
  * EPS is folded into the PSUM transpose: the sel matrix adds EPS*n_in to
    the s2 column, so (s2 + EPS*n_in)*rn = E[x^2] + EPS.
  * rsqrt: Quake seed + ONE Newton iteration (f32, DVE) — enough at this
    error budget.
  * DMA queues: weights on the SP HWDGE queue; the tiny collective bounce
    buffers (cc_in store / vals load) on the otherwise-idle Activation
    HWDGE queue; gpsimd triggers ONLY collectives.
  * DVE FIFO order per batch is arranged so mask derivation of the next
    batch runs inside the AllGather window: [derive q0,q1 | vstat |
    derive q2,q3 | PSUM copy | epilogue]. The two PSUM->SBUF staging
    copies run concurrently on DVE and the Activation engine, and the
    PSUM transpose is a single 34-row selector matmul per row block
    (stats rows at partitions 0:5, affine rows at 32:34).
"""

import numpy as np
import ml_dtypes

import concourse.bass as bass
import concourse.bacc as bacc
import concourse.tile as tile
import concourse.mybir as mybir
from concourse import bass_utils

L, W, NC = 8, 4096, 8
NB = L - 1                # 7 topo batches
RPC = W // NC             # 512 rows per core
CB = W // 128             # 32 contraction blocks of 128
RB = RPC // 128           # 4 row blocks of 128 per core
NQ = 4                    # c-block quarters per batch
QJ = CB // NQ             # 8 c-blocks per quarter
EPS = 1e-5

BF16 = mybir.dt.bfloat16
F32 = mybir.dt.float32
I32 = mybir.dt.int32
ADD = mybir.AluOpType.add
SUB = mybir.AluOpType.subtract
MUL = mybir.AluOpType.mult
NEQ = mybir.AluOpType.not_equal
RSHIFT = mybir.AluOpType.logical_shift_right

_CACHED = None


def _kernel_body(nc, tc, wm_d, xf_d, pf_d, sel34_d, y_d):
    with (
        tc.tile_pool(name="const", bufs=1) as constp,
        tc.tile_pool(name="wmp", bufs=8) as wmp,
        tc.tile_pool(name="mp", bufs=6) as mp,
        tc.tile_pool(name="vals", bufs=2) as valsp,
        tc.tile_pool(name="ep", bufs=2) as epp,
        tc.tile_pool(name="psum", bufs=1, space="PSUM") as psump,
        tc.tile_pool(name="dram", bufs=2, space="DRAM") as dramp,
    ):
        # ---- persistent: per-row params, folded [128, NB*5*RB] ----
        # s: 0=rn(1/n_in), 1=rowWM, 2=P0(gamma*gain), 3=P1((bias+beta*rowWM)*gain), 4=amp
        params = constp.tile([128, NB * 5 * RB], F32)
        nc.sync.dma_start(out=params[:], in_=pf_d.ap())

        def pslice(i, s):
            o = (i * 5 + s) * RB
            return params[:, o:o + RB]

        # row-selector matrix for the PSUM transpose (rows 0:5 = stats, rows
        # 32:34 = affine, matching the partition bases the copies can hit);
        # folds EPS*n_in into the s2 column
        sel34 = constp.tile([128, 8], F32, name="sel34")
        nc.sync.dma_start(out=sel34[0:34, :], in_=sel34_d.ap())

        # persistent staging tile for the transpose: rows 0:5 and 32:34 are
        # rewritten per batch; the other contracted rows must be 0 (not
        # garbage) so they contribute nothing through the zero sel rows
        sb_st = constp.tile([128, 512], F32, name="sb_st")
        nc.vector.memset(sb_st[:, :], 0.0)

        # ---- persistent: per-batch stationary vectors [128, CB*5] bf16 ----
        # col layout per c-block j: [vhi, vlo, sqhi, sqlo, ones]
        vstat = constp.tile([128, CB * 5], BF16)
        v5 = vstat[:].rearrange("p (j s) -> p j s", s=5)
        nc.vector.memset(v5[:, :, 4], 1.0)

        prev_cc_out = None
        for i in range(NB):
            # ======== weight streaming (vals-independent) + mask derive ====
            wm_t, m_t = [], []
            for q in range(NQ):
                wq = wmp.tile([128, QJ * RPC], BF16, tag="wm", name="wm")
                nc.sync.dma_start(
                    out=wq[:].rearrange("p (a b) -> p a b", b=RPC),
                    in_=wm_d[i][:, q * QJ:(q + 1) * QJ, :],
                )
                wm_t.append(wq)
            # derive m = (wm != 0) as bf16 0/1 on DVE. Quarters 0,1 emitted
            # before vstat (they run inside the previous batch's AllGather
            # window), 2,3 after (they run under this batch's sweep).
            for q in range(2):
                mq = mp.tile([128, QJ * RPC], BF16, tag="m", name="m")
                nc.vector.tensor_scalar(mq[:], wm_t[q][:], 0.0, None, op0=NEQ)
                m_t.append(mq)

            # ======== vals -> vstat ========================================
            vals = valsp.tile([128, CB], F32, tag="vals", name="vals")
            if i == 0:
                nc.scalar.dma_start(out=vals[:], in_=xf_d.ap())
            else:
                # cc payload is fold-major per rank: element (k, p, rb) = row
                # k*512 + rb*128 + p; c-block j = 4k + rb  =>  [p, (k rb)]
                nc.scalar.dma_start(
                    out=vals[:].rearrange("p (k rb) -> p k rb", rb=RB),
                    in_=prev_cc_out.rearrange("(k p rb) -> p k rb",
                                              p=128, rb=RB),
                )
            tmp_a = epp.tile([128, CB], F32, tag="vtmp_a", name="vtmp_a")
            tmp_sq = epp.tile([128, CB], F32, tag="vtmp_sq", name="vtmp_sq")
            nc.vector.tensor_copy(v5[:, :, 0], vals[:])             # vhi
            nc.vector.tensor_copy(tmp_a[:], v5[:, :, 0])
            nc.vector.tensor_tensor(v5[:, :, 1], vals[:], tmp_a[:], op=SUB)
            nc.vector.tensor_tensor(tmp_sq[:], vals[:], vals[:], op=MUL)
            nc.vector.tensor_copy(v5[:, :, 2], tmp_sq[:])           # sqhi
            nc.vector.tensor_copy(tmp_a[:], v5[:, :, 2])
            nc.vector.tensor_tensor(v5[:, :, 3], tmp_sq[:], tmp_a[:], op=SUB)

            for q in range(2, NQ):
                mq = mp.tile([128, QJ * RPC], BF16, tag="m", name="m")
                nc.vector.tensor_scalar(mq[:], wm_t[q][:], 0.0, None, op0=NEQ)
                m_t.append(mq)

            # ======== matvec sweep =========================================
            # ps_af rows: [WM@vhi, WM@vlo];  ps_st rows: [M@vhi, M@vlo,
            # M@sqhi, M@sqlo, n_in]
            ps_st = psump.tile([128, 512], F32, tag="ps_st", name="ps_st")
            ps_af = psump.tile([128, 512], F32, tag="ps_af", name="ps_af")
            for j in range(CB):
                q, jq = divmod(j, QJ)
                rhs_w = wm_t[q][:, jq * RPC:(jq + 1) * RPC]
                rhs_m = m_t[q][:, jq * RPC:(jq + 1) * RPC]
                st, sp = (j == 0), (j == CB - 1)
                nc.tensor.matmul(ps_af[0:2, :], lhsT=vstat[:, j * 5:j * 5 + 2],
                                 rhs=rhs_w, start=st, stop=sp)
                nc.tensor.matmul(ps_st[0:5, :], lhsT=vstat[:, j * 5:j * 5 + 5],
                                 rhs=rhs_m, start=st, stop=sp)

            # ======== transpose to fold layout =============================
            # copy PSUM row-groups to SBUF partitions 0:5 / 32:34 (engines
            # can only address partition bases 0/32/64/96), then per
            # row-block ONE matmul over rows 0:34 against sel34 lands
            # [128, 8] in PSUM: cols [s1, s2 + EPS*n_in, t1, 0...]
            nc.vector.tensor_copy(sb_st[0:5, :], ps_st[0:5, :])
            nc.scalar.activation(sb_st[32:34, :], ps_af[0:2, :],
                                 mybir.ActivationFunctionType.Copy)
            ps_t = psump.tile([128, RB * 512], F32, tag="ps_t", name="ps_t")
            for rb in range(RB):
                nc.tensor.matmul(
                    ps_t[:, rb * 512:rb * 512 + 8],
                    lhsT=sb_st[0:34, rb * 128:(rb + 1) * 128],
                    rhs=sel34[0:34, :], start=True, stop=True)
            pt3 = ps_t[:].rearrange("p (rb s) -> p rb s", s=512)

            # ======== epilogue (all [128, RB] f32) =========================
            def T(tag):
                return epp.tile([128, RB], F32, tag=tag, name=tag)

            # DVE reads at most one PSUM operand per instruction; pt3 cols:
            # 0=s1, 1=s2+EPS*n_in, 2=t1
            mean, ex2e, msq, vpe = T("mean"), T("ex2e"), T("msq"), T("vpe")
            nc.vector.tensor_tensor(mean[:], pt3[:, :, 0], pslice(i, 0), op=MUL)
            nc.vector.tensor_tensor(ex2e[:], pt3[:, :, 1], pslice(i, 0), op=MUL)
            nc.vector.tensor_tensor(msq[:], mean[:], mean[:], op=MUL)
            nc.vector.scalar_tensor_tensor(
                vpe[:], msq[:], -1.0, ex2e[:], op0=MUL, op1=ADD)
            # rs = 1/sqrt(vpe): Quake seed + 1 Newton iteration (f32, DVE)
            rs, nra, nrb = T("rs"), T("nra"), T("nrb")
            nc.vector.tensor_scalar(
                rs[:].bitcast(I32), vpe[:].bitcast(I32), 1, None, op0=RSHIFT)
            nc.vector.tensor_scalar(
                rs[:].bitcast(I32), rs[:].bitcast(I32), -1, 0x5F3759DF,
                op0=MUL, op1=ADD)
            nc.vector.tensor_tensor(nra[:], rs[:], rs[:], op=MUL)
            nc.vector.tensor_tensor(nrb[:], nra[:], vpe[:], op=MUL)
            nc.vector.tensor_scalar(nrb[:], nrb[:], -0.5, 1.5, op0=MUL, op1=ADD)
            nc.vector.tensor_tensor(rs[:], rs[:], nrb[:], op=MUL)
            # pre = P0*rs*(t1 - mean*rowWM) + P1
            mw, tm, g1g, pre = T("mw"), T("tm"), T("g1g"), T("pre")
            nc.vector.tensor_tensor(mw[:], mean[:], pslice(i, 1), op=MUL)
            nc.vector.tensor_tensor(tm[:], pt3[:, :, 2], mw[:], op=SUB)
            nc.vector.tensor_tensor(g1g[:], pslice(i, 2), rs[:], op=MUL)
            nc.vector.tensor_tensor(pre[:], g1g[:], tm[:], op=MUL)
            nc.vector.tensor_tensor(pre[:], pre[:], pslice(i, 3), op=ADD)
            outv = T("outv")
            if i < NB - 1:
                sil = T("sil")
                nc.scalar.activation(
                    sil[:], pre[:], mybir.ActivationFunctionType.Silu)
                nc.vector.tensor_tensor(outv[:], sil[:], pslice(i, 4), op=MUL)
            else:
                nc.vector.tensor_tensor(outv[:], pre[:], pslice(i, 4), op=MUL)

            # ======== scatter / all-gather =================================
            # payload is fold-major: cc_in[p*RB + rb] = outv[p, rb]
            # (contiguous 16B per partition). Bounce DMAs ride the idle
            # Activation HWDGE queue; gpsimd only triggers the collective.
            if i < NB - 1:
                cc_in = dramp.tile([RPC], F32, tag="cci", name="cci")
                cc_out = dramp.tile([W], F32, tag="cco", name="cco")
                nc.scalar.dma_start(
                    out=cc_in[:].rearrange("(p rb) -> p rb", rb=RB), in_=outv[:])
                nc.gpsimd.collective_compute(
                    "AllGather",
                    mybir.AluOpType.bypass,
                    replica_groups=[list(range(NC))],
                    ins=[cc_in[:].opt()],
                    outs=[cc_out[:].opt()],
                )
                prev_cc_out = cc_out
            else:
                nc.sync.dma_start(
                    out=y_d.ap().rearrange("(p rb) -> p rb", rb=RB), in_=outv[:])


def _build_program():
    nc = bacc.Bacc("TRN2", target_bir_lowering=False, debug=False,
                   num_devices=NC)
    wm_d = nc.dram_tensor("wm", [NB, 128, CB, RPC], BF16, kind="ExternalInput")
    xf_d = nc.dram_tensor("xf", [128, CB], F32, kind="ExternalInput")
    pf_d = nc.dram_tensor("pf", [128, NB * 5 * RB], F32, kind="ExternalInput")
    sel34_d = nc.dram_tensor("sel34", [34, 8], F32, kind="ExternalInput")
    y_d = nc.dram_tensor("y", [RPC], F32, kind="ExternalOutput")
    with tile.TileContext(nc) as tc:
        _kernel_body(nc, tc, wm_d, xf_d, pf_d, sel34_d, y_d)
    nc.compile()
    return nc


def _pack_inputs(x, weights, masks, biases, gamma, beta, gain, amplification):
    bf = ml_dtypes.bfloat16
    w32 = np.asarray(weights, np.float32)
    m32 = np.asarray(masks, np.float32)
    # premask with exact +0.0 at masked-out entries so the on-device
    # (wm != 0) mask derivation is exact
    wm = np.where(m32 != 0, w32, np.float32(0.0)).astype(bf)

    # input-independent per-row params
    n_in = m32.sum(axis=2, dtype=np.float32)                 # [NB, W]
    rowWM = wm.astype(np.float32).sum(axis=2)                # [NB, W]
    rn = (1.0 / n_in).astype(np.float32)
    gamma = np.asarray(gamma, np.float32).reshape(NB, W)
    beta = np.asarray(beta, np.float32).reshape(NB, W)
    biases = np.asarray(biases, np.float32).reshape(NB, W)
    gain = np.asarray(gain, np.float32).reshape(NB, W)
    amp = np.asarray(amplification, np.float32).reshape(NB, W)
    P0 = gamma * gain
    P1 = (biases + beta * rowWM) * gain

    # [NB, W(r), W(c)] -> [NB, p, jj, k, rr]  with r = k*RPC+rr, c = jj*128+p
    def fold(a):
        a = a.reshape(NB, NC, RPC, CB, 128)
        return a.transpose(0, 4, 3, 1, 2)

    wm_f = fold(wm)

    x32 = np.asarray(x, np.float32)
    xf = np.ascontiguousarray(x32.reshape(CB, 128).T)  # [128, CB]

    # params: [NB, W] -> [NB, NC, RB, 128] (row r = k*RPC + rb*128 + p)
    def fold_param(a):
        return np.ascontiguousarray(a, dtype=np.float32).reshape(NB, NC, RB, 128)

    ps = [fold_param(a) for a in (rn, rowWM, P0, P1, amp)]
    pall = np.stack(ps, axis=1)  # [NB, 5, NC, RB, 128]

    # transpose selector: sb rows [s1hi, s1lo, sqhi, sqlo, n_in] at 0:5 and
    # [t1hi, t1lo] at 32:34 -> cols [s1, s2 + EPS*n_in, t1, 0...]
    sel34 = np.zeros((34, 8), np.float32)
    sel34[0, 0] = sel34[1, 0] = 1.0
    sel34[2, 1] = sel34[3, 1] = 1.0
    sel34[4, 1] = EPS
    sel34[32, 2] = sel34[33, 2] = 1.0

    in_maps = []
    for k in range(NC):
        # pf[p, (i*5+s)*RB + rb]
        pf = np.ascontiguousarray(
            pall[:, :, k].transpose(3, 0, 1, 2).reshape(128, NB * 5 * RB))
        in_maps.append({
            "wm": wm_f[:, :, :, k, :],
            "xf": xf,
            "pf": pf,
            "sel34": sel34,
        })
    return in_maps


def _get_program():
    global _CACHED
    if _CACHED is None:
        _CACHED = _build_program()
    return _CACHED


def _run(in_maps, **kw):
    nc = _get_program()
    return bass_utils.run_bass_kernel_spmd(
        nc, in_maps, core_ids=list(range(NC)), **kw)


def _unfold_y(shard):
    # y[p*RB + rb] = out[rb*128 + p]
    return np.ascontiguousarray(
        np.asarray(shard, np.float32).reshape(128, RB).T.reshape(-1))


def kernel(x, weights, masks, biases, gamma, beta, gain, amplification):
    in_maps = _pack_inputs(x, weights, masks, biases, gamma, beta, gain,
                           amplification)
    res = _run(in_maps)
    return np.concatenate([_unfold_y(res.results[k]["y"]) for k in range(NC)])


def run_traced(inputs, **kw):
    """For test.py: same as kernel() but with NTFF profiling enabled."""
    in_maps = _pack_inputs(**inputs)
    res = _run(in_maps, trace=True, **kw)
    y = np.concatenate([_unfold_y(res.results[k]["y"]) for k in range(NC)])
    return y, res


# revision 48
# speedup vs baseline: 1.0509x; 1.0509x over previous
"""Trainium2 Bass kernel: topo-batched masked-norm NN forward (gnn_message_passing).

Math per topo batch i (reference.py):
    vals = previous layer activations [W]
    n_in[r]  = sum_c M[r,c]
    mean[r]  = (M @ vals)[r] / n_in[r]
    var[r]   = (M @ vals^2)[r] / n_in[r] - mean[r]^2
    rs[r]    = 1/sqrt(var[r] + EPS)
    affine[r]= gamma*rs*( (WM @ vals)[r] - mean[r]*rowWM[r] ) + beta[r]*rowWM[r]
               + bias[r],   WM = W ⊙ M,  rowWM = WM @ 1
    out = silu(affine*gain)*amp   (last batch: identity instead of silu)

Distribution: rows (output neurons) sharded across 8 cores (512 rows/core);
the 4096-vector of activations is all-gathered between batches.

Key layout/precision choices (vs the f32-accurate hi/lo variant):
  * ONE bf16 matrix per batch: WM = (W ⊙ M) premasked on host and shipped
    bf16 ([NB,128,CB,RPC], transposed: contraction dim on partitions).
    Halves HBM traffic vs hi/lo pairs + u8 masks; rel-err ~5.5e-3 vs the
    2e-2 gate (validated in numpy against the jax reference).
  * The 0/1 mask M is re-derived ON DEVICE as (WM != 0) on the DVE — exact,
    because no masked Gaussian weight rounds to bf16 zero, and the host
    writes +0.0 at masked-out entries. This keeps the gpsimd queue free
    (u8->bf16 converting DMA would require the slow software DGE, which
    delayed the collective trigger in the previous version).
  * Activations keep a bf16 hi/lo split (stationary side of the matmul is
    free: per-c-block cost is the 512 moving cols, not stationary rows).
  * n_in, 1/n_in, rowWM and the affine constants are input-independent =>
    precomputed on host and shipped as per-row params:
      P0 = gamma*gain,  P1 = (bias + beta*rowWM)*gain,  amp, rn=1/n_in, rowWM
    so the epilogue is ~15 DVE ops + one Silu ACT op per batch.
  * EPS is folded into the PSUM transpose: the sel matrix adds EPS*n_in to
    the s2 column, so (s2 + EPS*n_in)*rn = E[x^2] + EPS.
  * rsqrt: Quake seed + ONE Newton iteration (f32, DVE) — enough at this
    error budget.
  * DMA queues: weights on the SP HWDGE queue; the tiny collective bounce
    buffers (cc_in store / vals load) on the otherwise-idle Activation
    HWDGE queue; gpsimd triggers ONLY collectives.
  * DVE FIFO order per batch is arranged so mask derivation of the next
    batch runs inside the AllGather window: [derive q0,q1 | vstat |
    derive q2,q3 | PSUM copy | epilogue]. The two PSUM->SBUF staging
    copies run concurrently on DVE and the Activation engine, and the
    PSUM transpose is a single 34-row selector matmul per row block
    (stats rows at partitions 0:5, affine rows at 32:34).
"""

import numpy as np
import ml_dtypes

import concourse.bass as bass
import concourse.bacc as bacc
import concourse.tile as tile
import concourse.mybir as mybir
from concourse import bass_utils

L, W, NC = 8, 4096, 8
NB = L - 1                # 7 topo batches
RPC = W // NC             # 512 rows per core
CB = W // 128             # 32 contraction blocks of 128
RB = RPC // 128           # 4 row blocks of 128 per core
NQ = 4                    # c-block quarters per batch
QJ = CB // NQ             # 8 c-blocks per quarter
EPS = 1e-5

BF16 = mybir.dt.bfloat16
F8 = mybir.dt.float8e4
F32 = mybir.dt.float32
I32 = mybir.dt.int32
ADD = mybir.AluOpType.add
SUB = mybir.AluOpType.subtract
MUL = mybir.AluOpType.mult
NEQ = mybir.AluOpType.not_equal
RSHIFT = mybir.AluOpType.logical_shift_right
DR = mybir.MatmulPerfMode.DoubleRow

_CACHED = None


def _kernel_body(nc, tc, wm_d, xf_d, pf_d, sel34_d, y_d):
    with (
        tc.tile_pool(name="const", bufs=1) as constp,
        tc.tile_pool(name="wmp", bufs=8) as wmp,
        tc.tile_pool(name="mp", bufs=6) as mp,
        tc.tile_pool(name="vals", bufs=2) as valsp,
        tc.tile_pool(name="ep", bufs=2) as epp,
        tc.tile_pool(name="psum", bufs=1, space="PSUM") as psump,
        tc.tile_pool(name="dram", bufs=2, space="DRAM") as dramp,
    ):
        # ---- persistent: per-row params, folded [128, NB*5*RB] ----
        # s: 0=rn(1/n_in), 1=rowWM, 2=P0(gamma*gain), 3=P1((bias+beta*rowWM)*gain), 4=amp
        params = constp.tile([128, NB * 5 * RB], F32)
        nc.sync.dma_start(out=params[:], in_=pf_d.ap())

        def pslice(i, s):
            o = (i * 5 + s) * RB
            return params[:, o:o + RB]

        # row-selector matrix for the PSUM transpose (rows 0:5 = stats, rows
        # 32:34 = affine, matching the partition bases the copies can hit);
        # folds EPS*n_in into the s2 column
        sel34 = constp.tile([128, 8], F32, name="sel34")
        nc.sync.dma_start(out=sel34[0:34, :], in_=sel34_d.ap())

        # persistent staging tile for the transpose: rows 0:5 and 32:34 are
        # rewritten per batch; the other contracted rows must be 0 (not
        # garbage) so they contribute nothing through the zero sel rows
        sb_st = constp.tile([128, 512], F32, name="sb_st")
        nc.vector.memset(sb_st[:, :], 0.0)

        # ---- persistent: per-batch stationary vectors ----
        # bf16 pair for the affine pass: per c-block [vhi, vlo, sqhi, sqlo,
        # ones] (only cols 0:2 feed matmuls; 2:5 keep the build identical)
        vstat = constp.tile([128, CB * 5], BF16)
        v5 = vstat[:].rearrange("p (j s) -> p j s", s=5)
        nc.vector.memset(v5[:, :, 4], 1.0)
        # fp8 stationaries for the DoubleRow stats pass: [v8, sq8, ones,
        # pad*13] per c-block — padded to 16 so the dual-fp8 LDWEIGHTS pair
        # stride satisfies the ISA's step%16==0 requirement
        vstat8 = constp.tile([128, CB * 16], F8)
        v8 = vstat8[:].rearrange("p (j s) -> p j s", s=16)
        nc.vector.memset(v8[:, :, 2], 1.0)

        prev_cc_out = None
        for i in range(NB):
            # ======== weight streaming (vals-independent) + mask derive ====
            wm_t, m_t = [], []
            for q in range(NQ):
                wq = wmp.tile([128, QJ * RPC], BF16, tag="wm", name="wm")
                nc.sync.dma_start(
                    out=wq[:].rearrange("p (a b) -> p a b", b=RPC),
                    in_=wm_d[i][:, q * QJ:(q + 1) * QJ, :],
                )
                wm_t.append(wq)
            # derive m = (wm != 0) as fp8 0/1 on DVE, in HALF-quarter chunks
            # (fp8-output DVE ops run at half rate; short chunks limit how
            # long one can wedge into the epilogue via the engines' limited
            # out-of-order window). Quarters 0,1 emitted before vstat (they
            # run inside the previous batch's AllGather window), 2,3 after.
            HQ = QJ * RPC // 2
            for q in range(2):
                mq = mp.tile([128, QJ * RPC], F8, tag="m", name="m")
                for c in range(2):
                    nc.vector.tensor_scalar(
                        mq[:, c * HQ:(c + 1) * HQ],
                        wm_t[q][:, c * HQ:(c + 1) * HQ], 0.0, None, op0=NEQ)
                m_t.append(mq)

            # ======== vals -> vstat ========================================
            vals = valsp.tile([128, CB], F32, tag="vals", name="vals")
            if i == 0:
                nc.scalar.dma_start(out=vals[:], in_=xf_d.ap())
            else:
                # cc payload is fold-major per rank: element (k, p, rb) = row
                # k*512 + rb*128 + p; c-block j = 4k + rb  =>  [p, (k rb)]
                nc.scalar.dma_start(
                    out=vals[:].rearrange("p (k rb) -> p k rb", rb=RB),
                    in_=prev_cc_out.rearrange("(k p rb) -> p k rb",
                                              p=128, rb=RB),
                )
            tmp_a = epp.tile([128, CB], F32, tag="vtmp_a", name="vtmp_a")
            tmp_sq = epp.tile([128, CB], F32, tag="vtmp_sq", name="vtmp_sq")
            nc.vector.tensor_copy(v5[:, :, 0], vals[:])             # vhi
            nc.vector.tensor_copy(tmp_a[:], v5[:, :, 0])
            nc.vector.tensor_tensor(v5[:, :, 1], vals[:], tmp_a[:], op=SUB)
            nc.vector.tensor_tensor(tmp_sq[:], vals[:], vals[:], op=MUL)
            nc.vector.tensor_copy(v5[:, :, 2], tmp_sq[:])           # sqhi
            nc.vector.tensor_copy(tmp_a[:], v5[:, :, 2])
            nc.vector.tensor_tensor(v5[:, :, 3], tmp_sq[:], tmp_a[:], op=SUB)
            nc.vector.tensor_copy(v8[:, :, 0], vals[:])             # v8
            nc.vector.tensor_copy(v8[:, :, 1], tmp_sq[:])           # sq8

            for q in range(2, NQ):
                mq = mp.tile([128, QJ * RPC], F8, tag="m", name="m")
                for c in range(2):
                    nc.vector.tensor_scalar(
                        mq[:, c * HQ:(c + 1) * HQ],
                        wm_t[q][:, c * HQ:(c + 1) * HQ], 0.0, None, op0=NEQ)
                m_t.append(mq)

            # ======== matvec sweep =========================================
            # Two separate passes (one LDWEIGHTS mode switch per batch):
            # 1) affine, bf16: ps_af rows [WM@vhi, WM@vlo] — runs first and
            #    absorbs the PE clock ramp; its PSUM copy (on the Activation
            #    engine) overlaps pass 2.
            # 2) stats, fp8 DoubleRow (2 c-blocks per matmul at the same
            #    per-instruction cost): ps_st rows [M@v8, M@sq8, n_in]
            ps_st = psump.tile([128, 512], F32, tag="ps_st", name="ps_st")
            ps_af = psump.tile([128, 512], F32, tag="ps_af", name="ps_af")
            for j in range(CB):
                q, jq = divmod(j, QJ)
                nc.tensor.matmul(ps_af[0:2, :], lhsT=vstat[:, j * 5:j * 5 + 2],
                                 rhs=wm_t[q][:, jq * RPC:(jq + 1) * RPC],
                                 start=(j == 0), stop=(j == CB - 1))
            nc.scalar.activation(sb_st[32:34, :], ps_af[0:2, :],
                                 mybir.ActivationFunctionType.Copy)
            for u in range(CB // 2):
                q, uq = divmod(u, QJ // 2)
                o2 = 2 * uq * RPC
                vp = vstat8[:, u * 32:u * 32 + 32].rearrange(
                    "p (two s) -> p two s", s=16)
                nc.tensor.matmul(
                    ps_st[0:3, :], lhsT=vp[:, :, 0:3],
                    rhs=m_t[q][:, o2:o2 + 2 * RPC].rearrange(
                        "p (two n) -> p two n", two=2),
                    start=(u == 0), stop=(u == CB // 2 - 1), perf_mode=DR)

            # ======== transpose to fold layout =============================
            # copy PSUM row-groups to SBUF partitions 0:5 / 32:34 (engines
            # can only address partition bases 0/32/64/96), then per
            # row-block ONE matmul over rows 0:34 against sel34 lands
            # [128, 8] in PSUM: cols [s1, s2 + EPS*n_in, t1, 0...]
            nc.vector.tensor_copy(sb_st[0:3, :], ps_st[0:3, :])
            ps_t = psump.tile([128, RB * 512], F32, tag="ps_t", name="ps_t")
            for rb in range(RB):
                nc.tensor.matmul(
                    ps_t[:, rb * 512:rb * 512 + 8],
                    lhsT=sb_st[0:34, rb * 128:(rb + 1) * 128],
                    rhs=sel34[0:34, :], start=True, stop=True)
            pt3 = ps_t[:].rearrange("p (rb s) -> p rb s", s=512)

            # ======== epilogue (all [128, RB] f32) =========================
            def T(tag):
                return epp.tile([128, RB], F32, tag=tag, name=tag)

            # DVE reads at most one PSUM operand per instruction; pt3 cols:
            # 0=s1, 1=s2+EPS*n_in, 2=t1
            mean, ex2e, msq, vpe = T("mean"), T("ex2e"), T("msq"), T("vpe")
            nc.vector.tensor_tensor(mean[:], pt3[:, :, 0], pslice(i, 0), op=MUL)
            nc.vector.tensor_tensor(ex2e[:], pt3[:, :, 1], pslice(i, 0), op=MUL)
            nc.vector.tensor_tensor(msq[:], mean[:], mean[:], op=MUL)
            nc.vector.scalar_tensor_tensor(
                vpe[:], msq[:], -1.0, ex2e[:], op0=MUL, op1=ADD)
            # rs = 1/sqrt(vpe): Quake seed + 1 Newton iteration (f32, DVE)
            rs, nra, nrb = T("rs"), T("nra"), T("nrb")
            nc.vector.tensor_scalar(
                rs[:].bitcast(I32), vpe[:].bitcast(I32), 1, None, op0=RSHIFT)
            nc.vector.tensor_scalar(
                rs[:].bitcast(I32), rs[:].bitcast(I32), -1, 0x5F3759DF,
                op0=MUL, op1=ADD)
            nc.vector.tensor_tensor(nra[:], rs[:], rs[:], op=MUL)
            nc.vector.tensor_tensor(nrb[:], nra[:], vpe[:], op=MUL)
            nc.vector.tensor_scalar(nrb[:], nrb[:], -0.5, 1.5, op0=MUL, op1=ADD)
            nc.vector.tensor_tensor(rs[:], rs[:], nrb[:], op=MUL)
            # pre = P0*rs*(t1 - mean*rowWM) + P1
            mw, tm, g1g, pre = T("mw"), T("tm"), T("g1g"), T("pre")
            nc.vector.tensor_tensor(mw[:], mean[:], pslice(i, 1), op=MUL)
            nc.vector.tensor_tensor(tm[:], pt3[:, :, 2], mw[:], op=SUB)
            nc.vector.tensor_tensor(g1g[:], pslice(i, 2), rs[:], op=MUL)
            nc.vector.tensor_tensor(pre[:], g1g[:], tm[:], op=MUL)
            nc.vector.tensor_tensor(pre[:], pre[:], pslice(i, 3), op=ADD)
            outv = T("outv")
            if i < NB - 1:
                sil = T("sil")
                nc.scalar.activation(
                    sil[:], pre[:], mybir.ActivationFunctionType.Silu)
                nc.vector.tensor_tensor(outv[:], sil[:], pslice(i, 4), op=MUL)
            else:
                nc.vector.tensor_tensor(outv[:], pre[:], pslice(i, 4), op=MUL)

            # ======== scatter / all-gather =================================
            # payload is fold-major: cc_in[p*RB + rb] = outv[p, rb]
            # (contiguous 16B per partition). Bounce DMAs ride the idle
            # Activation HWDGE queue; gpsimd only triggers the collective.
            if i < NB - 1:
                cc_in = dramp.tile([RPC], F32, tag="cci", name="cci")
                cc_out = dramp.tile([W], F32, tag="cco", name="cco")
                nc.scalar.dma_start(
                    out=cc_in[:].rearrange("(p rb) -> p rb", rb=RB), in_=outv[:])
                nc.gpsimd.collective_compute(
                    "AllGather",
                    mybir.AluOpType.bypass,
                    replica_groups=[list(range(NC))],
                    ins=[cc_in[:].opt()],
                    outs=[cc_out[:].opt()],
                )
                prev_cc_out = cc_out
            else:
                nc.sync.dma_start(
                    out=y_d.ap().rearrange("(p rb) -> p rb", rb=RB), in_=outv[:])


def _build_program():
    nc = bacc.Bacc("TRN2", target_bir_lowering=False, debug=False,
                   num_devices=NC)
    wm_d = nc.dram_tensor("wm", [NB, 128, CB, RPC], BF16, kind="ExternalInput")
    xf_d = nc.dram_tensor("xf", [128, CB], F32, kind="ExternalInput")
    pf_d = nc.dram_tensor("pf", [128, NB * 5 * RB], F32, kind="ExternalInput")
    sel34_d = nc.dram_tensor("sel34", [34, 8], F32, kind="ExternalInput")
    y_d = nc.dram_tensor("y", [RPC], F32, kind="ExternalOutput")
    with tile.TileContext(nc) as tc:
        _kernel_body(nc, tc, wm_d, xf_d, pf_d, sel34_d, y_d)
    nc.compile()
    return nc


def _pack_inputs(x, weights, masks, biases, gamma, beta, gain, amplification):
    bf = ml_dtypes.bfloat16
    w32 = np.asarray(weights, np.float32)
    m32 = np.asarray(masks, np.float32)
    # premask with exact +0.0 at masked-out entries so the on-device
    # (wm != 0) mask derivation is exact
    wm = np.where(m32 != 0, w32, np.float32(0.0)).astype(bf)

    # input-independent per-row params
    n_in = m32.sum(axis=2, dtype=np.float32)                 # [NB, W]
    rowWM = wm.astype(np.float32).sum(axis=2)                # [NB, W]
    rn = (1.0 / n_in).astype(np.float32)
    gamma = np.asarray(gamma, np.float32).reshape(NB, W)
    beta = np.asarray(beta, np.float32).reshape(NB, W)
    biases = np.asarray(biases, np.float32).reshape(NB, W)
    gain = np.asarray(gain, np.float32).reshape(NB, W)
    amp = np.asarray(amplification, np.float32).reshape(NB, W)
    P0 = gamma * gain
    P1 = (biases + beta * rowWM) * gain

    # [NB, W(r), W(c)] -> [NB, p, jj, k, rr]  with r = k*RPC+rr, c = jj*128+p
    def fold(a):
        a = a.reshape(NB, NC, RPC, CB, 128)
        return a.transpose(0, 4, 3, 1, 2)

    wm_f = fold(wm)

    x32 = np.asarray(x, np.float32)
    xf = np.ascontiguousarray(x32.reshape(CB, 128).T)  # [128, CB]

    # params: [NB, W] -> [NB, NC, RB, 128] (row r = k*RPC + rb*128 + p)
    def fold_param(a):
        return np.ascontiguousarray(a, dtype=np.float32).reshape(NB, NC, RB, 128)

    ps = [fold_param(a) for a in (rn, rowWM, P0, P1, amp)]
    pall = np.stack(ps, axis=1)  # [NB, 5, NC, RB, 128]

    # transpose selector: sb rows [s1, s2, n_in] at 0:3 and [t1hi, t1lo] at
    # 32:34 -> cols [s1, s2 + EPS*n_in, t1, 0...]
    sel34 = np.zeros((34, 8), np.float32)
    sel34[0, 0] = 1.0
    sel34[1, 1] = 1.0
    sel34[2, 1] = EPS
    sel34[32, 2] = sel34[33, 2] = 1.0

    in_maps = []
    for k in range(NC):
        # pf[p, (i*5+s)*RB + rb]
        pf = np.ascontiguousarray(
            pall[:, :, k].transpose(3, 0, 1, 2).reshape(128, NB * 5 * RB))
        in_maps.append({
            "wm": wm_f[:, :, :, k, :],
            "xf": xf,
            "pf": pf,
            "sel34": sel34,
        })
    return in_maps


def _get_program():
    global _CACHED
    if _CACHED is None:
        _CACHED = _build_program()
    return _CACHED


def _run(in_maps, **kw):
    nc = _get_program()
    return bass_utils.run_bass_kernel_spmd(
        nc, in_maps, core_ids=list(range(NC)), **kw)


def _unfold_y(shard):
    # y[p*RB + rb] = out[rb*128 + p]
    return np.ascontiguousarray(
        np.asarray(shard, np.float32).reshape(128, RB).T.reshape(-1))


def kernel(x, weights, masks, biases, gamma, beta, gain, amplification):
    in_maps = _pack_inputs(x, weights, masks, biases, gamma, beta, gain,
                           amplification)
    res = _run(in_maps)
    return np.concatenate([_unfold_y(res.results[k]["y"]) for k in range(NC)])


def run_traced(inputs, **kw):
    """For test.py: same as kernel() but with NTFF profiling enabled."""
    in_maps = _pack_inputs(**inputs)
    res = _run(in_maps, trace=True, **kw)
    y = np.concatenate([_unfold_y(res.results[k]["y"]) for k in range(NC)])
    return y, res
